# revision 80
# baseline (speedup 1.0000x reference)
"""Llama GQA attention (B=4,S=1024,H=4096,NH=32,NKV=8,D=128) on 8 TRN2 cores.

Tensor-parallel over heads (4 q heads + 1 kv head per core); per-core partial
o_proj (row slice of w_o) summed on host (the all-reduce).

All on-device tensors are fp16 (fp32 PSUM accumulation), flowing in
transposed [feature, token] layout:
  qkv^T = w_qkv_slice.T @ hidden^T     (fp16 matmuls, K=H, m-outer chains)
  RoPE via stacked cos/sin tables      (DVE, 4 ops per 128x512 tile)
  scores^T[kv,sq] = kT.T @ qT          (K=D)
  probs = exp(scale*scores) * mask     (Act + DVE), fp16
  attn^T[d,sq] = v_nat.T @ probs       (v transposed on PE, K=kv)
  den  = partition_all_reduce(acc)     (GPSIMD), norm on DVE
  out^T = w_o_slice.T @ attn^T         (K=local hid, interleaved into the
                                        NEXT tile's attention as PE filler)

Per-core engine budget (TimelineSim cost model): PE busy ~611us is the fp16
matmul roofline (qkv 328 + o_proj 218 + causal-wedge attention 62);
Act/DVE/Pool/DMA all stay under ~250us and hide behind PE (~98% PE
utilization, ~624us wall vs 1153us baseline). fp8 was evaluated and fails
the 2e-2 accuracy gate by 3x; fp16 lands at ~7e-4.
"""

import numpy as np

B, S, H = 4, 1024, 4096
NH, NKV, D = 32, 8, 128
THETA = 10000.0
N_CORES = 8
NHL = NH // N_CORES            # 4 local q heads
TOK = B * S                    # 4096 tokens
NT = TOK // 512                # 8 token tiles
KT = H // 128                  # 32 contraction tiles for qkv
QKV_COLS = (NHL + 2) * D       # 768 local qkv columns (q0..q3 | k | v)
WO_K = NHL * D                 # 512 local o_proj contraction
SCALE = 1.0 / float(np.sqrt(D))

_PROG = {}


def _build_program():
    import concourse.mybir as mybir
    import concourse.tile as tile
    import concourse.bass_isa as bass_isa
    from concourse import bacc
    from concourse.masks import make_identity

    F16 = mybir.dt.float16
    F32 = mybir.dt.float32
    MUL = mybir.AluOpType.mult
    ADD = mybir.AluOpType.add
    SUB = mybir.AluOpType.subtract
    EXP = mybir.ActivationFunctionType.Exp
    RADD = bass_isa.ReduceOp.add

    nc = bacc.Bacc("TRN2", target_bir_lowering=False, debug=False,
                   num_devices=N_CORES)

    hT_d = nc.dram_tensor("hT", (H, TOK), F16, kind="ExternalInput")
    wq_d = nc.dram_tensor("wq", (H, QKV_COLS), F16, kind="ExternalInput")
    wo_d = nc.dram_tensor("wo", (WO_K, H), F16, kind="ExternalInput")
    cs_d = nc.dram_tensor("cs", (128, S), F16, kind="ExternalInput")
    sn_d = nc.dram_tensor("sn", (128, S), F16, kind="ExternalInput")
    mk_d = nc.dram_tensor("mk", (128, 128), F16, kind="ExternalInput")
    outT_d = nc.dram_tensor("outT", (H, TOK), F16, kind="ExternalOutput")

    hT_r = hT_d.rearrange("(ko ki) t -> ki ko t", ki=128)
    wq_r = wq_d.rearrange("(ko ki) c -> ki ko c", ki=128)
    wo_r = wo_d.rearrange("(kb ki) m -> ki kb m", ki=128)
    out_r = outT_d.rearrange("(mo mi) t -> mi mo t", mi=128)

    with nc.allow_low_precision(reason="fp16 activations"), \
         tile.TileContext(nc) as tc:
        with (
            tc.tile_pool(name="pp", bufs=1) as pp,
            tc.tile_pool(name="pd", bufs=2) as pd,
            tc.tile_pool(name="pps", bufs=2, space="PSUM") as pps,
        ):
            # ---- persistent tensors + prologue DMAs --------------------
            wq_sb = pp.tile([128, KT, QKV_COLS], F16)
            cs_sb = pp.tile([128, S], F16)
            sn_sb = pp.tile([128, S], F16)
            mk_sb = pp.tile([128, 128], F16)
            wo_sb = pp.tile([128, 4, H], F16)
            # warm tile via plain memset: ready ~0.5us before make_identity's
            # memset+affine-select chain, so PE warmup starts that much sooner
            warm = pp.tile([128, 128], F16)
            nc.gpsimd.memset(warm[:], 0.125)
            ident = pp.tile([128, 128], F16)
            make_identity(nc, ident[:])
            # exp bias: keeps probs in fp16 range (max raw score ~11.2);
            # cancels in softmax normalization
            biasc = pp.tile([128, 1], F32)
            nc.gpsimd.memset(biasc[:], -5.0)

            # PE warmup: dummy matmuls bridge the initial DMA wait so the
            # p-state ramp (3us of busy lead) completes before the first
            # real matmul; 40 ends right at the ramp + first-pair arrival
            for _ in range(40):
                wt = pps.tile([128, 128], F32, tag="sc", name="warm")
                nc.tensor.matmul(wt[:], warm[:], warm[:],
                                 start=True, stop=True)

            hts = {}

            def load_ht(n):
                # quarter-grain DMAs: a dma_start's dest completes atomically,
                # so finer slices unblock the first chain k-tiles sooner when
                # the queue is congested with the previous tile's stores
                lo = pd.tile([128, 16, 512], F16, tag="hT", bufs=3, name="htlo")
                hi = pd.tile([128, 16, 512], F16, tag="hT", bufs=3, name="hthi")
                for q in range(2):
                    nc.sync.dma_start(
                        lo[:, 8 * q:8 * q + 8, :],
                        hT_r[:, 8 * q:8 * q + 8, n * 512:(n + 1) * 512])
                for q in range(2):
                    nc.sync.dma_start(
                        hi[:, 8 * q:8 * q + 8, :],
                        hT_r[:, 16 + 8 * q:24 + 8 * q, n * 512:(n + 1) * 512])
                hts[n] = (lo, hi)

            # bootstrap: wq k-tiles and hT parts streamed as 2-2 pairs in
            # exact consumption order (kk ascending); per-kk supply (~1.0us)
            # is faster than PE demand (~1.3us) so DMA stays ahead all tile 0
            # (subtile deps gate per-kk reads)
            lo0 = pd.tile([128, 16, 512], F16, tag="hT", bufs=3, name="htlo")
            hi0 = pd.tile([128, 16, 512], F16, tag="hT", bufs=3, name="hthi")
            for j in range(8):
                nc.sync.dma_start(wq_sb[:, 2 * j:2 * j + 2, :],
                                  wq_r[:, 2 * j:2 * j + 2, :])
                nc.sync.dma_start(lo0[:, 2 * j:2 * j + 2, :],
                                  hT_r[:, 2 * j:2 * j + 2, 0:512])
            for j in range(8):
                nc.sync.dma_start(wq_sb[:, 16 + 2 * j:18 + 2 * j, :],
                                  wq_r[:, 16 + 2 * j:18 + 2 * j, :])
                nc.sync.dma_start(hi0[:, 2 * j:2 * j + 2, :],
                                  hT_r[:, 16 + 2 * j:18 + 2 * j, 0:512])
            hts[0] = (lo0, hi0)
            nc.sync.dma_start(cs_sb[:], cs_d[:])
            nc.sync.dma_start(sn_sb[:], sn_d[:])
            nc.sync.dma_start(mk_sb[:], mk_d[:])
            load_ht(1)
            # finer wo slices: the first po fillers (tile 1's attention)
            # need low-m columns right as the big ht1 transfers finish
            for wj in range(4):
                nc.sync.dma_start(wo_sb[:, :, wj * 1024:(wj + 1) * 1024],
                                  wo_r[:, :, wj * 1024:(wj + 1) * 1024])

            # ---- main loop over 512-token tiles ------------------------
            kT_cur = v_cur = None
            po_st = None   # o_proj state for tile n-1, drained inside tile n

            def emit_po_step(st):
                """One o_proj column tile: 4 chained matmuls + evict."""
                m = st["m"]
                if m >= 32:
                    return False
                if m % 8 == 0:
                    st["stage"] = pd.tile([128, 8, 512], F16, tag="st",
                                          name="stage")
                po_t = pps.tile([128, 512], F32, tag="po", bufs=3, name="po")
                for kb in range(4):
                    nc.tensor.matmul(po_t[:],
                                     wo_sb[:, kb, m * 128:(m + 1) * 128],
                                     st["attnT"][:, kb, :],
                                     start=(kb == 0), stop=(kb == 3))
                tsl_n = slice(st["n"] * 512, (st["n"] + 1) * 512)
                dst = st["stage"][:, m % 8, :]
                if m % 2 == 0:
                    nc.scalar.copy(dst, po_t[:])
                else:
                    nc.vector.tensor_copy(dst, po_t[:])
                if st["n"] == NT - 1 and m >= 27:
                    # tail: store per tile so the end drain flushes little
                    if m == 27:
                        nc.sync.dma_start(out_r[:, 24:28, tsl_n],
                                          st["stage"][:, 0:4, :])
                    else:
                        nc.sync.dma_start(out_r[:, m:m + 1, tsl_n],
                                          st["stage"][:, m % 8:m % 8 + 1, :])
                elif m % 8 == 7:
                    sblk = m // 8
                    nc.sync.dma_start(
                        out_r[:, 8 * sblk:8 * sblk + 8, tsl_n],
                        st["stage"][:])
                st["m"] += 1
                return True

            for n in range(NT):
                b, half = divmod(n, 2)
                tsl = slice(half * 512, (half + 1) * 512)
                lo, hi = hts.pop(n)
                if n + 1 < NT and n + 1 not in hts:
                    load_ht(n + 1)

                if half == 0:
                    kT_cur = pd.tile([128, S], F16, tag="kT", name="kT")
                    v_cur = pd.tile([128, 8, 128], F16, tag="vn", name="vn")
                qT_cur = pd.tile([128, NHL, 512], F16, tag="qT", name="qT")

                def chain(ps, cols):
                    for kk in range(KT):
                        src, ko = (lo, kk) if kk < 16 else (hi, kk - 16)
                        nc.tensor.matmul(ps[:], wq_sb[:, kk, cols],
                                         src[:, ko, :],
                                         start=(kk == 0), stop=(kk == KT - 1))

                ROLE_COLS = [slice(512, 640), slice(0, 128), slice(128, 256),
                             slice(256, 384), slice(384, 512), slice(640, 768)]

                def rope_epilogue(role, ps):
                    # RoPE straight off PSUM (partition-shifted reads are
                    # legal when one input is PSUM): cs_sb = [cos;cos],
                    # sn_sb = [-sin;sin]; out = x*cs + swap(x)*sn
                    ra = pd.tile([128, 512], F16, tag="ra", name="ra")
                    nc.vector.tensor_tensor(ra[:], ps[:], cs_sb[:, tsl],
                                            op=MUL)
                    rb = pd.tile([128, 512], F16, tag="rb", name="rb")
                    nc.vector.tensor_tensor(rb[0:64, :], ps[64:128, :],
                                            sn_sb[0:64, tsl], op=MUL)
                    nc.vector.tensor_tensor(rb[64:128, :], ps[0:64, :],
                                            sn_sb[64:128, tsl], op=MUL)
                    dest = (kT_cur[:, tsl] if role == 0
                            else qT_cur[:, role - 1, :])
                    nc.vector.tensor_tensor(dest, ra[:], rb[:], op=ADD)

                def v_epilogue(psv):
                    vT = pd.tile([128, 512], F16, tag="vT", name="vT")
                    nc.scalar.copy(vT[:], psv[:])
                    for c in range(4):
                        pt = pps.tile([128, 128], F16, tag="sc", name="ptv")
                        nc.tensor.transpose(pt[:], vT[:, c * 128:(c + 1) * 128],
                                            ident[:])
                        nc.vector.tensor_copy(v_cur[:, half * 4 + c, :], pt[:])

                if n == 0:
                    # prologue: all 6 chains k-part-interleaved across 6 psum
                    # banks (po/attn rings are idle until tile 1) so PE can
                    # consume weight/activation DMA parts as they arrive
                    PRO_TAGS = [("qm", None), ("qm", None), ("po", 3),
                                ("po", 3), ("po", 3), ("attn", 1)]
                    ptiles = []
                    for role in range(6):
                        tag, bf = PRO_TAGS[role]
                        ptiles.append(pps.tile([128, 512], F32, tag=tag,
                                               bufs=bf, name="pro"))
                    for part in range(4):
                        for role in range(6):
                            for kk in range(part * 8, part * 8 + 8):
                                src, ko = ((lo, kk) if kk < 16
                                           else (hi, kk - 16))
                                nc.tensor.matmul(
                                    ptiles[role][:],
                                    wq_sb[:, kk, ROLE_COLS[role]],
                                    src[:, ko, :],
                                    start=(kk == 0), stop=(kk == KT - 1))
                    for role in range(5):
                        rope_epilogue(role, ptiles[role])
                    v_epilogue(ptiles[5])
                else:
                    # steady state: k and first q head, then v (its transpose
                    # epilogue must clear before attention), then q1..q3
                    for role in (0, 1):
                        ps = pps.tile([128, 512], F32, tag="qm", name="psqk")
                        chain(ps, ROLE_COLS[role])
                        rope_epilogue(role, ps)
                    psv = pps.tile([128, 512], F32, tag="qm", name="psv")
                    chain(psv, ROLE_COLS[5])
                    v_epilogue(psv)
                    for role in (2, 3, 4):
                        ps = pps.tile([128, 512], F32, tag="qm", name="psqk")
                        chain(ps, ROLE_COLS[role])
                        rope_epilogue(role, ps)

                # --- attention h-loop (flattened, lookahead) with o_proj
                # of tile n-1 interleaved as PE filler ---
                jmax = 4 + half * 4
                attnT_cur = pd.tile([128, NHL, 512], F16, tag="attnT",
                                    name="attnT")
                steps = [(h, j) for h in range(NHL) for j in range(jmax)]
                LA = 3   # pr ring (4 bufs) holds LA+1 probs tiles in flight
                pend = {}
                attn_ps = {}
                acc_t = {}
                po_total = 32 if po_st is not None else 0
                po_done = 0

                def emit_s(h, j):
                    # causal wedge: diagonal kv blocks only see sq >= off
                    mi = j - half * 4
                    off = 128 * mi if mi > 0 else 0
                    width = 512 - off
                    ps_s = pps.tile([128, 512], F32, tag="sc", name="ps_s")
                    nc.tensor.matmul(ps_s[:, 0:width],
                                     kT_cur[:, j * 128:(j + 1) * 128],
                                     qT_cur[:, h, off:512],
                                     start=True, stop=True)
                    pr = pd.tile([128, 512], F16, tag="pr", bufs=4, name="pr")
                    nc.scalar.activation(pr[:, 0:width], ps_s[:, 0:width],
                                         EXP, scale=SCALE, bias=biasc[:])
                    if mi >= 0:
                        nc.vector.tensor_tensor(pr[:, 0:128], pr[:, 0:128],
                                                mk_sb[:], op=MUL)
                    pend[(h, j)] = (pr, off, width)

                def emit_a(h, j):
                    pr, off, width = pend.pop((h, j))
                    if j == 0:
                        attn_ps[h] = pps.tile([128, 512], F32, tag="attn",
                                              bufs=1, name="ps_attn")
                        acc_t[h] = pd.tile([128, 512], F16, tag="acc",
                                           name="acc")
                    nc.tensor.matmul(attn_ps[h][:, off:512], v_cur[:, j, :],
                                     pr[:, 0:width],
                                     start=(j == 0), stop=(j == jmax - 1))
                    if j == 0:
                        nc.vector.tensor_copy(acc_t[h][:], pr[:])
                    else:
                        nc.vector.tensor_tensor(acc_t[h][:, off:512],
                                                acc_t[h][:, off:512],
                                                pr[:, 0:width], op=ADD)
                    if j == jmax - 1:
                        denb = pd.tile([128, 512], F16, tag="den", name="denb")
                        nc.gpsimd.partition_all_reduce(denb[:], acc_t[h][:],
                                                       channels=128,
                                                       reduce_op=RADD)
                        recipb = pd.tile([128, 512], F16, tag="rec",
                                         name="recipb")
                        nc.vector.reciprocal(recipb[:], denb[:])
                        nc.vector.tensor_tensor(attnT_cur[:, h, :],
                                                attn_ps[h][:], recipb[:],
                                                op=MUL)

                for idx, (h, j) in enumerate(steps):
                    emit_s(h, j)
                    burst = 0
                    if idx >= LA:
                        ah, aj = steps[idx - LA]
                        emit_a(ah, aj)
                        if aj == jmax - 1:
                            # extra filler while the den-chain drains so the
                            # next head's attn bank frees in time
                            burst = 4
                    if po_st is not None:
                        want = min(po_total,
                                   po_total * (idx + 2) // len(steps) + burst)
                        if n == NT - 1:
                            # reserve filler for the den-chain tail latency
                            want = min(want, po_total - 5)
                        while po_done < want and emit_po_step(po_st):
                            po_done += 1
                for idx in range(max(0, len(steps) - LA), len(steps)):
                    emit_a(*steps[idx])
                while po_st is not None and emit_po_step(po_st):
                    pass

                po_st = {"attnT": attnT_cur, "n": n, "m": 0, "stage": None}

            # tail: o_proj of the last tile
            while emit_po_step(po_st):
                pass

    nc.compile()
    return nc


def _get_program():
    if "nc" not in _PROG:
        _PROG["nc"] = _build_program()
    return _PROG["nc"]


def _host_inputs(positions, hidden_states, w_qkv, w_o):
    positions = np.asarray(positions)
    hidden = np.asarray(hidden_states, dtype=np.float32)
    w_qkv = np.asarray(w_qkv, dtype=np.float32)
    w_o = np.asarray(w_o, dtype=np.float32)

    hT = np.ascontiguousarray(hidden.reshape(TOK, H).T).astype(np.float16)

    pos0 = positions[0].astype(np.float32)
    inv = 1.0 / (THETA ** (np.arange(64, dtype=np.float32) / 64.0))
    ang = inv[:, None] * pos0[None, :]            # [64, S]
    cos = np.cos(ang).astype(np.float32)
    sin = np.sin(ang).astype(np.float32)
    cs = np.concatenate([cos, cos], axis=0).astype(np.float16)   # [cos;cos]
    sn = np.concatenate([-sin, sin], axis=0).astype(np.float16)  # [-sin;sin]

    p = np.arange(128)[:, None]
    f = np.arange(128)[None, :]
    mk = (p <= f).astype(np.float16)

    in_maps = []
    for c in range(N_CORES):
        q0 = c * NHL * D
        kc = NH * D + c * D
        vc = NH * D + NKV * D + c * D
        wq = np.ascontiguousarray(np.concatenate(
            [w_qkv[:, q0:q0 + NHL * D],
             w_qkv[:, kc:kc + D],
             w_qkv[:, vc:vc + D]], axis=1)).astype(np.float16)
        wo = np.ascontiguousarray(
            w_o[c * WO_K:(c + 1) * WO_K, :]).astype(np.float16)
        in_maps.append({"hT": hT, "wq": wq, "wo": wo, "cs": cs, "sn": sn,
                        "mk": mk})
    return in_maps


def run(positions, hidden_states, w_qkv, w_o, trace=False):
    from concourse import bass_utils
    nc = _get_program()
    in_maps = _host_inputs(positions, hidden_states, w_qkv, w_o)
    res = bass_utils.run_bass_kernel_spmd(
        nc, in_maps, core_ids=list(range(N_CORES)), trace=trace)
    acc = np.zeros((H, TOK), dtype=np.float32)
    for c in range(N_CORES):
        acc += res.results[c]["outT"].astype(np.float32)
    out = np.ascontiguousarray(acc.T).reshape(B, S, H)
    return out, res


def kernel(positions, hidden_states, w_qkv, w_o):
    out, _ = run(positions, hidden_states, w_qkv, w_o, trace=False)
    return out



# revision 81
# speedup vs baseline: 1.0038x; 1.0038x over previous
"""Llama GQA attention (B=4,S=1024,H=4096,NH=32,NKV=8,D=128) on 8 TRN2 cores.

Tensor-parallel over heads (4 q heads + 1 kv head per core); per-core partial
o_proj (row slice of w_o) summed on host (the all-reduce).

All on-device tensors are fp16 (fp32 PSUM accumulation), flowing in
transposed [feature, token] layout:
  qkv^T = w_qkv_slice.T @ hidden^T     (fp16 matmuls, K=H, m-outer chains)
  RoPE via stacked cos/sin tables      (DVE, 4 ops per 128x512 tile)
  scores^T[kv,sq] = kT.T @ qT          (K=D)
  probs = exp(scale*scores) * mask     (Act + DVE), fp16
  attn^T[d,sq] = v_nat.T @ probs       (v transposed on PE, K=kv)
  den  = partition_all_reduce(acc)     (GPSIMD), norm on DVE
  out^T = w_o_slice.T @ attn^T         (K=local hid, interleaved into the
                                        NEXT tile's attention as PE filler)

Per-core engine budget (TimelineSim cost model): PE busy ~611us is the fp16
matmul roofline (qkv 328 + o_proj 218 + causal-wedge attention 62);
Act/DVE/Pool/DMA all stay under ~250us and hide behind PE (~98% PE
utilization, ~624us wall vs 1153us baseline). fp8 was evaluated and fails
the 2e-2 accuracy gate by 3x; fp16 lands at ~7e-4.
"""

import numpy as np

B, S, H = 4, 1024, 4096
NH, NKV, D = 32, 8, 128
THETA = 10000.0
N_CORES = 8
NHL = NH // N_CORES            # 4 local q heads
TOK = B * S                    # 4096 tokens
NT = TOK // 512                # 8 token tiles
KT = H // 128                  # 32 contraction tiles for qkv
QKV_COLS = (NHL + 2) * D       # 768 local qkv columns (q0..q3 | k | v)
WO_K = NHL * D                 # 512 local o_proj contraction
SCALE = 1.0 / float(np.sqrt(D))

_PROG = {}


def _build_program():
    import concourse.mybir as mybir
    import concourse.tile as tile
    import concourse.bass_isa as bass_isa
    from concourse import bacc
    from concourse.masks import make_identity

    F16 = mybir.dt.float16
    F32 = mybir.dt.float32
    MUL = mybir.AluOpType.mult
    ADD = mybir.AluOpType.add
    SUB = mybir.AluOpType.subtract
    EXP = mybir.ActivationFunctionType.Exp
    RADD = bass_isa.ReduceOp.add

    nc = bacc.Bacc("TRN2", target_bir_lowering=False, debug=False,
                   num_devices=N_CORES)

    hT_d = nc.dram_tensor("hT", (H, TOK), F16, kind="ExternalInput")
    wq_d = nc.dram_tensor("wq", (H, QKV_COLS), F16, kind="ExternalInput")
    wo_d = nc.dram_tensor("wo", (WO_K, H), F16, kind="ExternalInput")
    cs_d = nc.dram_tensor("cs", (128, S), F16, kind="ExternalInput")
    sn_d = nc.dram_tensor("sn", (128, S), F16, kind="ExternalInput")
    mk_d = nc.dram_tensor("mk", (128, 128), F16, kind="ExternalInput")
    outT_d = nc.dram_tensor("outT", (H, TOK), F16, kind="ExternalOutput")

    hT_r = hT_d.rearrange("(ko ki) t -> ki ko t", ki=128)
    wq_r = wq_d.rearrange("(ko ki) c -> ki ko c", ki=128)
    wo_r = wo_d.rearrange("(kb ki) m -> ki kb m", ki=128)
    out_r = outT_d.rearrange("(mo mi) t -> mi mo t", mi=128)

    with nc.allow_low_precision(reason="fp16 activations"), \
         tile.TileContext(nc) as tc:
        with (
            tc.tile_pool(name="pp", bufs=1) as pp,
            tc.tile_pool(name="pd", bufs=2) as pd,
            tc.tile_pool(name="pps", bufs=2, space="PSUM") as pps,
        ):
            # ---- persistent tensors + prologue DMAs --------------------
            wq_sb = pp.tile([128, KT, QKV_COLS], F16)
            cs_sb = pp.tile([128, S], F16)
            sn_sb = pp.tile([128, S], F16)
            mk_sb = pp.tile([128, 128], F16)
            wo_sb = pp.tile([128, 4, H], F16)
            # warm tile via plain memset: ready ~0.5us before make_identity's
            # memset+affine-select chain, so PE warmup starts that much sooner
            warm = pp.tile([128, 128], F16)
            nc.gpsimd.memset(warm[:], 0.125)
            ident = pp.tile([128, 128], F16)
            make_identity(nc, ident[:])
            # exp bias: keeps probs in fp16 range (max raw score ~11.2);
            # cancels in softmax normalization
            biasc = pp.tile([128, 1], F32)
            nc.gpsimd.memset(biasc[:], -5.0)

            # PE warmup: dummy matmuls bridge the initial DMA wait so the
            # p-state ramp (3us of busy lead) completes before the first
            # real matmul; 40 ends right at the ramp + first-pair arrival
            for _ in range(40):
                wt = pps.tile([128, 128], F32, tag="sc", name="warm")
                nc.tensor.matmul(wt[:], warm[:], warm[:],
                                 start=True, stop=True)

            hts = {}

            def load_ht(n):
                # quarter-grain DMAs: a dma_start's dest completes atomically,
                # so finer slices unblock the first chain k-tiles sooner when
                # the queue is congested with the previous tile's stores
                lo = pd.tile([128, 16, 512], F16, tag="hT", bufs=3, name="htlo")
                hi = pd.tile([128, 16, 512], F16, tag="hT", bufs=3, name="hthi")
                for q in range(2):
                    nc.sync.dma_start(
                        lo[:, 8 * q:8 * q + 8, :],
                        hT_r[:, 8 * q:8 * q + 8, n * 512:(n + 1) * 512])
                for q in range(2):
                    nc.sync.dma_start(
                        hi[:, 8 * q:8 * q + 8, :],
                        hT_r[:, 16 + 8 * q:24 + 8 * q, n * 512:(n + 1) * 512])
                hts[n] = (lo, hi)

            # bootstrap: wq k-tiles and hT parts streamed as 2-2 pairs in
            # exact consumption order (kk ascending); per-kk supply (~1.0us)
            # is faster than PE demand (~1.3us) so DMA stays ahead all tile 0
            # (subtile deps gate per-kk reads)
            lo0 = pd.tile([128, 16, 512], F16, tag="hT", bufs=3, name="htlo")
            hi0 = pd.tile([128, 16, 512], F16, tag="hT", bufs=3, name="hthi")
            for j in range(8):
                nc.sync.dma_start(wq_sb[:, 2 * j:2 * j + 2, :],
                                  wq_r[:, 2 * j:2 * j + 2, :])
                nc.sync.dma_start(lo0[:, 2 * j:2 * j + 2, :],
                                  hT_r[:, 2 * j:2 * j + 2, 0:512])
            for j in range(8):
                nc.sync.dma_start(wq_sb[:, 16 + 2 * j:18 + 2 * j, :],
                                  wq_r[:, 16 + 2 * j:18 + 2 * j, :])
                nc.sync.dma_start(hi0[:, 2 * j:2 * j + 2, :],
                                  hT_r[:, 16 + 2 * j:18 + 2 * j, 0:512])
            hts[0] = (lo0, hi0)
            nc.sync.dma_start(cs_sb[:], cs_d[:])
            nc.sync.dma_start(sn_sb[:], sn_d[:])
            nc.sync.dma_start(mk_sb[:], mk_d[:])
            load_ht(1)
            # finer wo slices: the first po fillers (tile 1's attention)
            # need low-m columns right as the big ht1 transfers finish
            for wj in range(4):
                nc.sync.dma_start(wo_sb[:, :, wj * 1024:(wj + 1) * 1024],
                                  wo_r[:, :, wj * 1024:(wj + 1) * 1024])

            # ---- main loop over 512-token tiles ------------------------
            kT_cur = v_cur = None
            po_st = None   # o_proj state for tile n-1, drained inside tile n

            def emit_po_step(st):
                """One o_proj column tile: 4 chained matmuls + evict."""
                m = st["m"]
                if m >= 32:
                    return False
                if m % 8 == 0:
                    st["stage"] = pd.tile([128, 8, 512], F16, tag="st",
                                          name="stage")
                po_t = pps.tile([128, 512], F32, tag="po", bufs=3, name="po")
                for kb in range(4):
                    nc.tensor.matmul(po_t[:],
                                     wo_sb[:, kb, m * 128:(m + 1) * 128],
                                     st["attnT"][:, kb, :],
                                     start=(kb == 0), stop=(kb == 3))
                tsl_n = slice(st["n"] * 512, (st["n"] + 1) * 512)
                dst = st["stage"][:, m % 8, :]
                if m % 2 == 0:
                    nc.scalar.copy(dst, po_t[:])
                else:
                    nc.vector.tensor_copy(dst, po_t[:])
                if st["n"] == NT - 1 and m >= 27:
                    # tail: store per tile so the end drain flushes little
                    if m == 27:
                        nc.sync.dma_start(out_r[:, 24:28, tsl_n],
                                          st["stage"][:, 0:4, :])
                    else:
                        nc.sync.dma_start(out_r[:, m:m + 1, tsl_n],
                                          st["stage"][:, m % 8:m % 8 + 1, :])
                elif m % 8 == 7:
                    sblk = m // 8
                    nc.sync.dma_start(
                        out_r[:, 8 * sblk:8 * sblk + 8, tsl_n],
                        st["stage"][:])
                st["m"] += 1
                return True

            for n in range(NT):
                b, half = divmod(n, 2)
                tsl = slice(half * 512, (half + 1) * 512)
                lo, hi = hts.pop(n)
                if n + 1 < NT and n + 1 not in hts:
                    load_ht(n + 1)

                if half == 0:
                    kT_cur = pd.tile([128, S], F16, tag="kT", name="kT")
                    v_cur = pd.tile([128, 8, 128], F16, tag="vn", name="vn")
                qT_cur = pd.tile([128, NHL, 512], F16, tag="qT", name="qT")

                def chain(ps, cols):
                    for kk in range(KT):
                        src, ko = (lo, kk) if kk < 16 else (hi, kk - 16)
                        nc.tensor.matmul(ps[:], wq_sb[:, kk, cols],
                                         src[:, ko, :],
                                         start=(kk == 0), stop=(kk == KT - 1))

                ROLE_COLS = [slice(512, 640), slice(0, 128), slice(128, 256),
                             slice(256, 384), slice(384, 512), slice(640, 768)]

                def rope_epilogue(role, ps):
                    # RoPE straight off PSUM (partition-shifted reads are
                    # legal when one input is PSUM): cs_sb = [cos;cos],
                    # sn_sb = [-sin;sin]; out = x*cs + swap(x)*sn
                    ra = pd.tile([128, 512], F16, tag="ra", name="ra")
                    nc.vector.tensor_tensor(ra[:], ps[:], cs_sb[:, tsl],
                                            op=MUL)
                    rb = pd.tile([128, 512], F16, tag="rb", name="rb")
                    nc.vector.tensor_tensor(rb[0:64, :], ps[64:128, :],
                                            sn_sb[0:64, tsl], op=MUL)
                    nc.vector.tensor_tensor(rb[64:128, :], ps[0:64, :],
                                            sn_sb[64:128, tsl], op=MUL)
                    dest = (kT_cur[:, tsl] if role == 0
                            else qT_cur[:, role - 1, :])
                    nc.vector.tensor_tensor(dest, ra[:], rb[:], op=ADD)

                def v_epilogue(psv):
                    # transpose to natural [token, d] layout on the DMA xbar
                    # (14ns/32x32 tile) instead of PE+DVE; issued ~3 chains
                    # before attention consumes v, so latency hides fully
                    vT = pd.tile([128, 512], F16, tag="vT", name="vT")
                    nc.scalar.copy(vT[:], psv[:])
                    for c in range(4):
                        nc.sync.dma_start_transpose(
                            v_cur[:, half * 4 + c, :],
                            vT[:, c * 128:(c + 1) * 128])

                if n == 0:
                    # prologue: all 6 chains k-part-interleaved across 6 psum
                    # banks (po/attn rings are idle until tile 1) so PE can
                    # consume weight/activation DMA parts as they arrive
                    PRO_TAGS = [("qm", None), ("qm", None), ("po", 3),
                                ("po", 3), ("po", 3), ("attn", 1)]
                    ptiles = []
                    for role in range(6):
                        tag, bf = PRO_TAGS[role]
                        ptiles.append(pps.tile([128, 512], F32, tag=tag,
                                               bufs=bf, name="pro"))
                    for part in range(4):
                        for role in range(6):
                            for kk in range(part * 8, part * 8 + 8):
                                src, ko = ((lo, kk) if kk < 16
                                           else (hi, kk - 16))
                                nc.tensor.matmul(
                                    ptiles[role][:],
                                    wq_sb[:, kk, ROLE_COLS[role]],
                                    src[:, ko, :],
                                    start=(kk == 0), stop=(kk == KT - 1))
                    for role in range(5):
                        rope_epilogue(role, ptiles[role])
                    v_epilogue(ptiles[5])
                else:
                    # steady state: k and first q head, then v (its transpose
                    # epilogue must clear before attention), then q1..q3
                    for role in (0, 1):
                        ps = pps.tile([128, 512], F32, tag="qm", name="psqk")
                        chain(ps, ROLE_COLS[role])
                        rope_epilogue(role, ps)
                    psv = pps.tile([128, 512], F32, tag="qm", name="psv")
                    chain(psv, ROLE_COLS[5])
                    v_epilogue(psv)
                    for role in (2, 3, 4):
                        ps = pps.tile([128, 512], F32, tag="qm", name="psqk")
                        chain(ps, ROLE_COLS[role])
                        rope_epilogue(role, ps)

                # --- attention h-loop (flattened, lookahead) with o_proj
                # of tile n-1 interleaved as PE filler ---
                jmax = 4 + half * 4
                attnT_cur = pd.tile([128, NHL, 512], F16, tag="attnT",
                                    name="attnT")
                steps = [(h, j) for h in range(NHL) for j in range(jmax)]
                LA = 3   # pr ring (4 bufs) holds LA+1 probs tiles in flight
                pend = {}
                attn_ps = {}
                acc_t = {}
                po_total = 32 if po_st is not None else 0
                po_done = 0

                def emit_s(h, j):
                    # causal wedge: diagonal kv blocks only see sq >= off
                    mi = j - half * 4
                    off = 128 * mi if mi > 0 else 0
                    width = 512 - off
                    ps_s = pps.tile([128, 512], F32, tag="sc", name="ps_s")
                    nc.tensor.matmul(ps_s[:, 0:width],
                                     kT_cur[:, j * 128:(j + 1) * 128],
                                     qT_cur[:, h, off:512],
                                     start=True, stop=True)
                    pr = pd.tile([128, 512], F16, tag="pr", bufs=4, name="pr")
                    nc.scalar.activation(pr[:, 0:width], ps_s[:, 0:width],
                                         EXP, scale=SCALE, bias=biasc[:])
                    if mi >= 0:
                        nc.vector.tensor_tensor(pr[:, 0:128], pr[:, 0:128],
                                                mk_sb[:], op=MUL)
                    pend[(h, j)] = (pr, off, width)

                def emit_a(h, j):
                    pr, off, width = pend.pop((h, j))
                    if j == 0:
                        attn_ps[h] = pps.tile([128, 512], F32, tag="attn",
                                              bufs=1, name="ps_attn")
                        acc_t[h] = pd.tile([128, 512], F16, tag="acc",
                                           name="acc")
                    nc.tensor.matmul(attn_ps[h][:, off:512], v_cur[:, j, :],
                                     pr[:, 0:width],
                                     start=(j == 0), stop=(j == jmax - 1))
                    if j == 0:
                        nc.vector.tensor_copy(acc_t[h][:], pr[:])
                    else:
                        nc.vector.tensor_tensor(acc_t[h][:, off:512],
                                                acc_t[h][:, off:512],
                                                pr[:, 0:width], op=ADD)
                    if j == jmax - 1:
                        denb = pd.tile([128, 512], F16, tag="den", name="denb")
                        nc.gpsimd.partition_all_reduce(denb[:], acc_t[h][:],
                                                       channels=128,
                                                       reduce_op=RADD)
                        recipb = pd.tile([128, 512], F16, tag="rec",
                                         name="recipb")
                        nc.vector.reciprocal(recipb[:], denb[:])
                        nc.vector.tensor_tensor(attnT_cur[:, h, :],
                                                attn_ps[h][:], recipb[:],
                                                op=MUL)

                for idx, (h, j) in enumerate(steps):
                    emit_s(h, j)
                    burst = 0
                    if idx >= LA:
                        ah, aj = steps[idx - LA]
                        emit_a(ah, aj)
                        if aj == jmax - 1:
                            # extra filler while the den-chain drains so the
                            # next head's attn bank frees in time
                            burst = 4
                    if po_st is not None:
                        want = min(po_total,
                                   po_total * (idx + 2) // len(steps) + burst)
                        if n == NT - 1:
                            # reserve filler for the den-chain tail latency
                            want = min(want, po_total - 5)
                        while po_done < want and emit_po_step(po_st):
                            po_done += 1
                for idx in range(max(0, len(steps) - LA), len(steps)):
                    emit_a(*steps[idx])
                while po_st is not None and emit_po_step(po_st):
                    pass

                po_st = {"attnT": attnT_cur, "n": n, "m": 0, "stage": None}

            # tail: o_proj of the last tile
            while emit_po_step(po_st):
                pass

    nc.compile()
    return nc


def _get_program():
    if "nc" not in _PROG:
        _PROG["nc"] = _build_program()
    return _PROG["nc"]


def _host_inputs(positions, hidden_states, w_qkv, w_o):
    positions = np.asarray(positions)
    hidden = np.asarray(hidden_states, dtype=np.float32)
    w_qkv = np.asarray(w_qkv, dtype=np.float32)
    w_o = np.asarray(w_o, dtype=np.float32)

    hT = np.ascontiguousarray(hidden.reshape(TOK, H).T).astype(np.float16)

    pos0 = positions[0].astype(np.float32)
    inv = 1.0 / (THETA ** (np.arange(64, dtype=np.float32) / 64.0))
    ang = inv[:, None] * pos0[None, :]            # [64, S]
    cos = np.cos(ang).astype(np.float32)
    sin = np.sin(ang).astype(np.float32)
    cs = np.concatenate([cos, cos], axis=0).astype(np.float16)   # [cos;cos]
    sn = np.concatenate([-sin, sin], axis=0).astype(np.float16)  # [-sin;sin]

    p = np.arange(128)[:, None]
    f = np.arange(128)[None, :]
    mk = (p <= f).astype(np.float16)

    in_maps = []
    for c in range(N_CORES):
        q0 = c * NHL * D
        kc = NH * D + c * D
        vc = NH * D + NKV * D + c * D
        wq = np.ascontiguousarray(np.concatenate(
            [w_qkv[:, q0:q0 + NHL * D],
             w_qkv[:, kc:kc + D],
             w_qkv[:, vc:vc + D]], axis=1)).astype(np.float16)
        wo = np.ascontiguousarray(
            w_o[c * WO_K:(c + 1) * WO_K, :]).astype(np.float16)
        in_maps.append({"hT": hT, "wq": wq, "wo": wo, "cs": cs, "sn": sn,
                        "mk": mk})
    return in_maps


def run(positions, hidden_states, w_qkv, w_o, trace=False):
    from concourse import bass_utils
    nc = _get_program()
    in_maps = _host_inputs(positions, hidden_states, w_qkv, w_o)
    res = bass_utils.run_bass_kernel_spmd(
        nc, in_maps, core_ids=list(range(N_CORES)), trace=trace)
    acc = np.zeros((H, TOK), dtype=np.float32)
    for c in range(N_CORES):
        acc += res.results[c]["outT"].astype(np.float32)
    out = np.ascontiguousarray(acc.T).reshape(B, S, H)
    return out, res


def kernel(positions, hidden_states, w_qkv, w_o):
    out, _ = run(positions, hidden_states, w_qkv, w_o, trace=False)
    return out

